# revision 21
# baseline (speedup 1.0000x reference)
"""Trainium2 Bass kernel for additive (Bahdanau) attention — fp8 DoubleRow.

reference:
    proj_f = features @ W1_w + W1_b          # [B, L, ATT]
    proj_h = (hidden @ W2_w + W2_b)[:, None] # [B, 1, ATT]
    scores = tanh(proj_f + proj_h) @ V_w + V_b   # [B, L]
    alpha  = softmax(scores, axis=1)
    context = einsum('bl,ble->be', alpha, features)
    returns (alpha, context)

Sharding: data-parallel over batch B=64 across 8 cores (8 examples/core).
Weights replicated. No collectives.

Per-core algorithm (X = 8 examples), evolved from the bf16 baseline:
  - main GEMM and V-dot run in fp8e4 with perf_mode=DoubleRow (2 fp8
    weights/PE cell, K=256 per matmul).  To dodge e4m3 subnormals,
    features are scaled x32 and W1/V x64; the tanh activation descales
    via its scale arg (2^-11) and exp via scale 1/64 (softmax is
    shift-invariant so V_b is dropped, and the max-subtract works on the
    x64 scores with a rescaled bias).
  - context needs bf16 features for accuracy (fp8 features give ~3e-2
    rel err), so it runs on PE against the NATURAL bf16 tiles; only one
    transposed copy (ft8, fp8 x32) is kept, written by the DVE directly
    from the transpose PSUM with the scale fused.
  - software pipeline per example x's 16 GEMM blocks:
      loop top    : feature DMA issue for x+3 (half 1 at block 8)
      every block : one DVE f32->bf16 cast for x+2 (DMAs landed an
                    example earlier, so casts never wait)
      blocks 4-6  : deferred context stages of example x-1 on PE
      blocks 8-15 : PE transposes for x+1 (2 groups of 4 per block),
                    each group copied PSUM->ft8 with the x32 fp8 scale
  - scores accumulate in PSUM [1, 512] via DoubleRow V-dot matmuls that
    trail the tanh by two blocks so the PE never waits on ACT.
  - no-max softmax: scores are bounded (|s| ~< 2) so exp is applied
    straight from the two score PSUM banks (half 0 already at block 10);
    context uses the UNNORMALIZED exp via PE matmuls against the natural
    bf16 tiles (lhsT = transposed exp column), with 1/sum folded into
    the final PSUM->SBUF copies; alpha output is exp * 1/sum.
"""

import numpy as np

B, L, ENC, DEC, ATT = 64, 1024, 1024, 1024, 1024
N_CORES = 8
X = B // N_CORES  # examples per core
P = 128
NE = ENC // P  # 8
NA = ATT // P  # 8
ND = DEC // P  # 8
LH = 512       # free-dim half for fp32 PSUM bank
NL = L // LH   # 2

FSCALE = 32.0                      # feature scale into fp8
WSCALE = 64.0                      # W1 / V scale into fp8
PSCALE = 1.0 / (FSCALE * WSCALE)   # descale inside tanh
SSCALE = 1.0 / WSCALE              # descale inside exp

_CACHE = {}


def _build():
    import concourse.bacc as bacc
    import concourse.mybir as mybir
    import concourse.tile as tile

    f32, bf16 = mybir.dt.float32, mybir.dt.bfloat16
    fp8 = mybir.dt.float8e4
    Tanh = mybir.ActivationFunctionType.Tanh
    Exp = mybir.ActivationFunctionType.Exp
    DR = mybir.MatmulPerfMode.DoubleRow
    mult = mybir.AluOpType.mult
    AX = mybir.AxisListType.X

    nc = bacc.Bacc("TRN2", target_bir_lowering=False, debug=False, num_devices=N_CORES)

    feats = nc.declare_dram_parameter("features", [X, L, ENC], f32, isOutput=False)
    hid = nc.declare_dram_parameter("hidden_state", [X, DEC], f32, isOutput=False)
    w1 = nc.declare_dram_parameter("W1_w", [ENC, ATT], f32, isOutput=False)
    w1b = nc.declare_dram_parameter("W1_b", [ATT], f32, isOutput=False)
    w2 = nc.declare_dram_parameter("W2_w", [DEC, ATT], f32, isOutput=False)
    w2b = nc.declare_dram_parameter("W2_b", [ATT], f32, isOutput=False)
    vw = nc.declare_dram_parameter("V_w", [ATT], f32, isOutput=False)
    alpha_o = nc.declare_dram_parameter("alpha", [X, L], f32, isOutput=True)
    ctx_o = nc.declare_dram_parameter("context", [X, ENC], f32, isOutput=True)

    eye_dram = nc.inline_tensor(np.eye(P, dtype=np.float32), "eye128")

    with tile.TileContext(nc) as tc:
        with (
            tc.tile_pool(name="const", bufs=1) as const,
            tc.tile_pool(name="fn", bufs=14) as fnp,
            tc.tile_pool(name="fb", bufs=24) as fbp,
            tc.tile_pool(name="f8", bufs=2) as f8p,
            tc.tile_pool(name="mm", bufs=3, space="PSUM") as psum,
            tc.tile_pool(name="sc", bufs=3, space="PSUM") as spsum,
            tc.tile_pool(name="tp", bufs=2, space="PSUM") as tpsum,
            tc.tile_pool(name="tb", bufs=6) as tp,
            tc.tile_pool(name="al", bufs=2) as alp,
            tc.tile_pool(name="ms", bufs=1) as ms,
        ):
            # ---------------- prep: constants & weights ----------------
            eye = const.tile([P, P], f32, tag="eye")
            nc.sync.dma_start(eye[:], eye_dram[:, :])
            eye_bf = const.tile([P, P], bf16, tag="eye_bf")
            nc.vector.tensor_copy(eye_bf[:], eye[:])

            # small prep loads first so they never queue behind feature loads
            h_nat = alp.tile([X, DEC], f32, tag="esb", name="h_nat")
            nc.sync.dma_start(h_nat[:], hid[:, :])
            b1_nat = alp.tile([1, ATT], f32, tag="scores", name="b1_nat")
            nc.sync.dma_start(b1_nat[:], w1b[None, :])
            b2_nat = alp.tile([1, ATT], f32, tag="esb", name="b2_nat")
            nc.sync.dma_start(b2_nat[:], w2b[None, :])
            v_nat = alp.tile([1, ATT], f32, tag="scores", name="v_nat")
            nc.sync.dma_start(v_nat[:], vw[None, :])

            # W1 -> fp8 x64 in DoubleRow pair layout: w1d[q][p, i, m] =
            # 64*W1[(2q+i)*128 + p, m]
            w1d = []
            for q in range(NE // 2):
                t = const.tile([P, 2, ATT], fp8, tag=f"w1d_{q}")
                w1d.append(t)
            w1_stage = []
            for e in range(NE):
                stage = fnp.tile([P, ATT], f32, tag="fn", name=f"w1s{e}")
                nc.sync.dma_start(stage[:], w1[P * e : P * (e + 1), :])
                w1_stage.append(stage)

            w2t = []
            for e in range(ND):
                t = const.tile([P, ATT], bf16, tag=f"w2_{e}")
                w2t.append(t)

            # ---------------- per-example staging helpers ----------------
            fnat_map = {}   # (x, c) -> f32 natural tile
            fb_map = {}     # x -> {c: bf16 natural tile}
            f8_map = {}     # x -> ft8 tile  [P, NE*L] fp8 (features x32)

            def emit_dma_half(x, half):
                for c in range(4 * half, 4 * half + 4):
                    fnat = fnp.tile([P, ENC], f32, tag="fn", name=f"fn{x}_{c}")
                    fnat_map[(x, c)] = fnat
                    for q2 in range(4):
                        nc.sync.dma_start(
                            fnat[:, 256 * q2 : 256 * (q2 + 1)],
                            feats[x, P * c : P * (c + 1), 256 * q2 : 256 * (q2 + 1)],
                        )

            def emit_cast(x, k):
                # k = 2*c + hh : cast half hh of chunk c, f32 -> bf16 on DVE
                c, hh = divmod(k, 2)
                if hh == 0:
                    fb_map.setdefault(x, {})[c] = fbp.tile([P, ENC], bf16, tag="fb", name=f"fb{x}_{c}")
                fb = fb_map[x][c]
                nc.vector.tensor_copy(
                    fb[:, LH * hh : LH * (hh + 1)],
                    fnat_map[(x, c)][:, LH * hh : LH * (hh + 1)],
                )
                if hh == 1:
                    fnat_map.pop((x, c))

            def ft_view(ft):
                return ft.rearrange("p (e lc c) -> p e lc c", e=NE, lc=NE)

            def emit_transpose_group(x, g):
                # g = 2*lc + h : transpose blocks (e in [4h, 4h+4), l-chunk lc),
                # then one DVE copy PSUM bf16 -> ft8 fp8 with the x32 scale
                lc, h = divmod(g, 2)
                if g == 0:
                    f8_map[x] = f8p.tile([P, NE * L], fp8, tag="f8", name=f"f8{x}")
                f8 = f8_map[x]
                fb = fb_map[x][lc]
                tps = tpsum.tile([P, 4 * P], bf16, tag="tp")
                for j in range(4):
                    e = 4 * h + j
                    nc.tensor.transpose(
                        tps[:, P * j : P * (j + 1)],
                        fb[:, P * e : P * (e + 1)],
                        eye_bf[:],
                    )
                dst = ft_view(f8)[:, 4 * h : 4 * h + 4, lc, :]
                s2 = tps.rearrange("p (e c) -> p e c", e=4)
                if h == 0:
                    nc.vector.tensor_scalar_mul(dst, s2, FSCALE)
                else:
                    nc.scalar.activation(
                        dst, s2, mybir.ActivationFunctionType.Identity, scale=FSCALE
                    )

            # ---------------- prologue ----------------
            emit_dma_half(0, 0)
            for st in w1_stage:
                pass  # W1 stage DMAs were issued above, right after x0 half 0
            emit_dma_half(0, 1)
            w2_stage = []
            for e in range(ND):
                stg = fnp.tile([P, ATT], f32, tag="fn", name=f"w2s{e}")
                nc.sync.dma_start(stg[:], w2[P * e : P * (e + 1), :])
                w2_stage.append(stg)
            for k in range(16):
                emit_cast(0, k)
            # W1 -> fp8 on ACT, parallel with the DVE feature casts
            Identity = mybir.ActivationFunctionType.Identity
            for e in range(NE):
                q, i = divmod(e, 2)
                nc.scalar.activation(
                    w1d[q][:, i, :], w1_stage[e][:], Identity, scale=WSCALE
                )
            emit_dma_half(1, 0)
            emit_dma_half(1, 1)

            # first half of example 0's transposes (gates the first GEMM block)
            for g in range(8):
                emit_transpose_group(0, g)
            # W2 -> bf16 on DVE (fast HW-queue f32 loads replace the slow
            # gpsimd casting DMA, which gated the whole prologue)
            for e in range(ND):
                nc.vector.tensor_copy(w2t[e][:], w2_stage[e][:])

            # hT_all[p, c, x] = hid[x, 128c + p] via natural load + PE transpose
            hn_bf = ms.tile([X, DEC], bf16, tag="hn_bf")
            nc.vector.tensor_copy(hn_bf[:], h_nat[:])
            hTb = ms.tile([P, ND, X], bf16, tag="hTb")
            for c in range(ND):
                tps_h = tpsum.tile([P, X], bf16, tag="tp", name=f"tpsh{c}")
                nc.tensor.transpose(tps_h[:], hn_bf[:, P * c : P * (c + 1)], eye_bf[0:X, 0:X])
                nc.vector.tensor_copy(hTb[:, c, :], tps_h[:])

            # bias / V vectors transposed into [128, NA] (partition = within-chunk)
            def load_transposed_vec(nat, name):
                tps_v = tpsum.tile([P, NA], f32, tag="tp", name=f"tps_{name}")
                for c in range(NA):
                    nc.tensor.transpose(
                        tps_v[:, c : c + 1], nat[:, P * c : P * (c + 1)], eye[0:1, 0:1]
                    )
                dst = ms.tile([P, NA], f32, tag=name, name=name)
                nc.vector.tensor_copy(dst[:], tps_v[:])
                return dst

            b1T = load_transposed_vec(b1_nat, "b1T")
            b2T = load_transposed_vec(b2_nat, "b2T")
            vT = load_transposed_vec(v_nat, "vT")
            bT = ms.tile([P, NA], f32, tag="bT")
            nc.vector.tensor_add(bT[:], b1T[:], b2T[:])

            # V in fp8 x64 DoubleRow pair layout: vwd[p, i, j] = 64*V[(2j+i)*128+p]
            # (free-dim padded to 16 so the Ko step is 16 B)
            vwd = ms.tile([P, 2, 16], fp8, tag="vwd")
            nc.vector.tensor_scalar_mul(
                vwd[:, :, 0:4], vT.rearrange("p (j two) -> p two j", two=2), WSCALE
            )

            # proj_h transposed, plus bias: phb[p, a, x]
            phb = ms.tile([P, NA, X], f32, tag="phb")
            for a in range(NA):
                ph_ps = psum.tile([P, X], f32, tag="mm")
                for e in range(ND):
                    nc.tensor.matmul(
                        ph_ps[:],
                        w2t[e][:, P * a : P * (a + 1)],
                        hTb[:, e, :],
                        start=(e == 0),
                        stop=(e == ND - 1),
                    )
                nc.vector.tensor_scalar_add(phb[:, a, :], ph_ps[:], bT[:, a : a + 1])

            # second half of example 0's transposes, example 1 casts,
            # example 2 feature prefetch
            for g in range(8, 16):
                emit_transpose_group(0, g)
            emit_dma_half(2, 0)
            emit_dma_half(2, 1)
            for k in range(16):
                emit_cast(1, k)

            # ---------------- main per-example pipeline ----------------
            pending = []

            def flush_pending(cur_b=10**6):
                keep = []
                for sc_ap, j, tb_ap, b_emit in pending:
                    if b_emit <= cur_b - 2:
                        nc.tensor.matmul(
                            sc_ap,
                            vwd[:, :, j : j + 1],
                            tb_ap,
                            start=(j == 0),
                            stop=(j == 3),
                            perf_mode=DR,
                        )
                    else:
                        keep.append((sc_ap, j, tb_ap, b_emit))
                pending[:] = keep

            pending_ctx = []

            def flush_ctx(n):
                for _ in range(min(n, len(pending_ctx))):
                    pending_ctx.pop(0)()

            for x in range(X):
                cast_for = x + 2 if x + 2 < X else None
                trans_for = x + 1 if x + 1 < X else None
                dma_for = x + 3 if x + 3 < X else None
                if dma_for is not None:
                    emit_dma_half(dma_for, 0)

                ft8v = f8_map[x].rearrange("p (e l) -> p e l", e=NE)
                sc_h = {}
                tb3 = None
                for b in range(16):
                    lh, a = divmod(b, 8)
                    j, i = divmod(a, 2)
                    if a == 0:
                        sc_h[lh] = spsum.tile([1, LH], f32, tag="sc", name=f"sch{x}_{lh}")
                    if i == 0:
                        tb3 = tp.tile([P, 2, LH], fp8, tag="tb")
                    pp = psum.tile([P, LH], f32, tag="mm")
                    for q in range(4):
                        nc.tensor.matmul(
                            pp[:],
                            w1d[q][:, :, P * a : P * (a + 1)],
                            ft8v[:, 2 * q : 2 * q + 2, LH * lh : LH * (lh + 1)],
                            start=(q == 0),
                            stop=(q == 3),
                            perf_mode=DR,
                        )
                        if q == 1:
                            flush_pending(b)

                    nc.scalar.activation(
                        tb3[:, i, :], pp[:], Tanh,
                        bias=phb[:, a, x : x + 1], scale=PSCALE,
                    )
                    if i == 1:
                        pending.append((sc_h[lh][:], j, tb3[:], b))

                    # deferred context matmuls of example x-1, behind the
                    # softmax latency
                    if 4 <= b < 8:
                        flush_ctx(1)
                    # scheduled pipeline work for later examples
                    if dma_for is not None and b == 8:
                        emit_dma_half(dma_for, 1)
                    if cast_for is not None and b < 8:
                        emit_cast(cast_for, 2 * b)
                        emit_cast(cast_for, 2 * b + 1)
                    if trans_for is not None and b >= 8:
                        emit_transpose_group(trans_for, 2 * (b - 8))
                        emit_transpose_group(trans_for, 2 * (b - 8) + 1)

                    if b == 9:
                        # scores half 0 is complete (trail-2 flush at b9):
                        # unnormalized exp straight from PSUM
                        esb = alp.tile([1, L], f32, tag="esb", name=f"esb{x}")
                        ssum0 = alp.tile([1, 1], f32, tag="ssum0")
                        nc.scalar.activation(
                            esb[:, 0:LH], sc_h[0][:], Exp, scale=SSCALE,
                            accum_out=ssum0[:],
                        )

                flush_pending()

                # finish the no-max softmax: exp of half 1, sum, reciprocal
                ssum1 = alp.tile([1, 1], f32, tag="ssum1")
                nc.scalar.activation(
                    esb[:, LH:L], sc_h[1][:], Exp, scale=SSCALE,
                    accum_out=ssum1[:],
                )
                ssum = alp.tile([1, 1], f32, tag="ssum")
                nc.vector.tensor_add(ssum[:], ssum0[:], ssum1[:])
                rinv = alp.tile([1, 1], f32, tag="rinv")
                nc.vector.reciprocal(rinv[:], ssum[:])
                a32 = alp.tile([1, L], f32, tag="scores", name=f"a32_{x}")
                nc.vector.tensor_scalar_mul(a32[:], esb[:], rinv[:])
                nc.sync.dma_start(alpha_o[x, :], a32[:])

                # context on PE against the natural bf16 tiles, deferred into
                # x+1's early blocks.  Uses the UNNORMALIZED exp (esb) so it
                # only waits on the exps, not on a32; 1/sum lands in the final
                # PSUM->SBUF copies.
                def make_ctx(x, esb, rinv):
                    fb_x = fb_map[x]
                    alT = alp.tile([P, NE], bf16, tag="alT", name=f"alT{x}")
                    ctr2 = alp.tile([1, ENC], f32, tag="ctr2", name=f"ctr2_{x}")
                    state = {}

                    def stage0():
                        tps_a = tpsum.tile([P, NE], f32, tag="tp", name=f"tpsa{x}")
                        for lc in range(NE):
                            nc.tensor.transpose(
                                tps_a[:, lc : lc + 1], esb[:, P * lc : P * (lc + 1)],
                                eye[0:1, 0:1],
                            )
                        nc.vector.tensor_copy(alT[:], tps_a[:])

                    def half(eh):
                        def run():
                            cps = psum.tile([1, LH], f32, tag="mm", name=f"cps{x}_{eh}")
                            state[eh] = cps
                            for lc in range(NE):
                                nc.tensor.matmul(
                                    cps[:],
                                    alT[:, lc : lc + 1],
                                    fb_x[lc][:, LH * eh : LH * (eh + 1)],
                                    start=(lc == 0),
                                    stop=(lc == NE - 1),
                                )
                            nc.vector.tensor_scalar_mul(
                                ctr2[:, LH * eh : LH * (eh + 1)], state[eh][:], rinv[:]
                            )
                            if eh == 1:
                                nc.sync.dma_start(ctx_o[x, :], ctr2[:])
                        return run

                    return [stage0, half(0), half(1)]

                pending_ctx.extend(make_ctx(x, esb, rinv))
                if x == X - 1:
                    flush_ctx(len(pending_ctx))

    nc.compile()
    return nc


def kernel(features, hidden_state, W1_w, W1_b, W2_w, W2_b, V_w, V_b):
    from concourse.bass_utils import run_bass_kernel_spmd

    if "nc" not in _CACHE:
        _CACHE["nc"] = _build()
    nc = _CACHE["nc"]

    features = np.ascontiguousarray(np.asarray(features, dtype=np.float32))
    hidden_state = np.ascontiguousarray(np.asarray(hidden_state, dtype=np.float32))
    W1_w = np.ascontiguousarray(np.asarray(W1_w, dtype=np.float32))
    W1_b = np.ascontiguousarray(np.asarray(W1_b, dtype=np.float32))
    W2_w = np.ascontiguousarray(np.asarray(W2_w, dtype=np.float32))
    W2_b = np.ascontiguousarray(np.asarray(W2_b, dtype=np.float32))
    V_w = np.ascontiguousarray(np.asarray(V_w, dtype=np.float32))

    in_maps = []
    for c in range(N_CORES):
        in_maps.append(
            {
                "features": np.ascontiguousarray(features[c * X : (c + 1) * X]),
                "hidden_state": np.ascontiguousarray(hidden_state[c * X : (c + 1) * X]),
                "W1_w": W1_w,
                "W1_b": W1_b,
                "W2_w": W2_w,
                "W2_b": W2_b,
                "V_w": V_w,
            }
        )

    res = run_bass_kernel_spmd(nc, in_maps, list(range(N_CORES)), **_CACHE.get("run_kwargs", {}))
    _CACHE["last_result"] = res
    alpha = np.concatenate([res.results[c]["alpha"] for c in range(N_CORES)], axis=0)
    context = np.concatenate([res.results[c]["context"] for c in range(N_CORES)], axis=0)
    return alpha, context
